# revision 14
# baseline (speedup 1.0000x reference)
"""Causal self-attention (B=4, T=2048, C=1024, H=16) on 8 trn2 NeuronCores.

Sharding: core = (batch b, head-half s).  Each core computes q/k/v
projections for its 8 heads (weights pre-sliced/transposed on host),
causal flash-style attention with transposed score tiles, and a partial
(row-sharded) c_proj.  Host gather sums the two partials per batch.

HAM note: the PE clock-gate (K/N throttle) watches array activity; the
attention matmuls are padded to look full-array (128-row zero-padded K
stationaries, 128-col V stationaries) so the PE stays at 2.4 GHz.

Device data layout (mdt = bf16 by default):
  xT    [1024, 2048]  x[b].T                      (in-ch on partitions)
  wqkT  [1024, 1024]  [Wq_local | Wk_local].T     (in-ch on partitions)
  bqk   [128, 8]      q/k bias, per out-ch block (f32)
  wvT   [1024, 512]   Wv_local.T
  wpT   [512, 1024]   Wproj[:, local].T
  bpj   [128, 8]      bproj + bv@WprojT (folded), half per core (f32)
  maskT [128, 128]    upper-tri keep mask (j >= p), for diagonal blocks
  zT    [1024, 2048]  partial output, transposed (mdt; host upcasts + sums)
"""

import os
import sys

sys.path.insert(0, "/opt/trn_rl_repo")

import numpy as np

B, T, C, H = 4, 2048, 1024, 16
D = 64          # head dim
NH = 8          # heads per core
LC = NH * D     # local channels = 512
P = 128
QT = 512        # query tile (also matmul moving free dim)
NQT = T // QT   # 4
NKB = T // P    # 16 key blocks
IC = C // P     # 8 input-channel blocks
VW = D + 1      # per-head v columns (v | ones)

# matmul input dtype: bfloat16 streams the PE at full rate; float32r is
# exact-ish but half-rate on hw.
MM_DT = os.environ.get("BASS_ATTN_MM_DT", "bfloat16")

_nc_cache = {}


def _build_nc():
    from contextlib import ExitStack

    import concourse.bass as bass  # noqa: F401
    import concourse.mybir as mybir
    from concourse import bacc, tile

    f32 = mybir.dt.float32
    mdt = getattr(mybir.dt, MM_DT)
    Exp = mybir.ActivationFunctionType.Exp
    Copy = mybir.ActivationFunctionType.Copy
    Ident = mybir.ActivationFunctionType.Identity

    nc = bacc.Bacc("TRN2", target_bir_lowering=False, debug=False, num_devices=8)
    xT = nc.dram_tensor("xT", [C, T], mdt, kind="ExternalInput").ap()
    wqkT = nc.dram_tensor("wqkT", [C, 2 * LC], mdt, kind="ExternalInput").ap()
    bqk = nc.dram_tensor("bqk", [P, 2 * LC // P], f32, kind="ExternalInput").ap()
    wvT = nc.dram_tensor("wvT", [C, LC], mdt, kind="ExternalInput").ap()
    wpT = nc.dram_tensor("wpT", [LC, C], mdt, kind="ExternalInput").ap()
    bpj = nc.dram_tensor("bpj", [P, C // P], f32, kind="ExternalInput").ap()
    maskT = nc.dram_tensor("maskT", [P, P], mdt, kind="ExternalInput").ap()
    zT = nc.dram_tensor("zT", [C, T], mdt, kind="ExternalOutput").ap()

    with tile.TileContext(nc) as tc:
        with ExitStack() as outer:
            persist = outer.enter_context(tc.tile_pool(name="persist", bufs=1))
            # q_sb[i]: [2 heads x 64 dims, tok 2048]
            q_sb = [persist.tile([P, T], mdt, tag=f"q{i}", name=f"q{i}") for i in range(4)]
            # kpad_sb[h]: per-head K with the other head-half zeroed, so the
            # score matmul stationary is a full 128-row block (HAM-warm)
            kpad_sb = [persist.tile([P, T], mdt, tag=f"kp{h}", name=f"kp{h}") for h in range(NH)]
            # v_sb[kb]: flat [tok 128, 8*(64+1) + pad]; head h at cols 65h;
            # col 65h+64 is the ones column (softmax denominator row)
            v_sb = [persist.tile([P, NH * VW + D + 1], mdt, tag=f"v{i}", name=f"v{i}")
                    for i in range(NKB)]
            bqk_sb = persist.tile([P, 8], f32, tag="bqk")
            bpj_sb = persist.tile([P, 8], f32, tag="bpj")
            mask_sb = persist.tile([P, P], mdt, tag="mask")
            # softmax sum/recip staging: heads live at 32-aligned rows
            # (3 groups x rows {0,32,64}) so the broadcast-matmul rhs has a
            # legal partition base
            sg_sb = [persist.tile([P, QT], f32, tag=f"sg{g}", name=f"sg{g}") for g in range(3)]
            rg_sb = [persist.tile([P, QT], f32, tag=f"rg{g}", name=f"rg{g}") for g in range(3)]
            rbf_sb = [persist.tile([P, QT], mdt, tag=f"rb{g}", name=f"rb{g}") for g in range(3)]
            ones_sb = persist.tile([P, D], mdt, tag="ones")

            # zero the dead half of each kpad tile / the v tail pad; overlaps
            # with the initial DMA wait
            for h in range(NH):
                half = slice(D, P) if h % 2 == 0 else slice(0, D)
                nc.vector.memset(kpad_sb[h][half, :], 0.0)
            for i in range(NKB):
                nc.vector.memset(v_sb[i][:, NH * VW:], 0.0)
            for g in range(3):
                nc.vector.memset(sg_sb[g][:], 1.0)

            nc.sync.dma_start(bqk_sb[:], bqk)
            nc.sync.dma_start(bpj_sb[:], bpj)
            nc.sync.dma_start(mask_sb[:], maskT)

            # ---- Stage A/B: qk projection + v projection, streaming x ----
            with tc.tile_pool(name="wts", bufs=1) as wpool, \
                 tc.tile_pool(name="xs", bufs=2) as xpool, \
                 tc.tile_pool(name="psab", bufs=4, space="PSUM") as pspool:
                wqk_sb = [wpool.tile([P, 2 * LC], mdt, tag=f"wqk{i}", name=f"wqk{i}") for i in range(IC)]
                wv_sb = [wpool.tile([P, LC], mdt, tag=f"wv{i}", name=f"wv{i}") for i in range(IC)]
                # DMA order: first the (wqk, x@tt0) pairs the opening qk-proj
                # needs, then wv; gets the PE streaming within ~2us
                xt0 = [xpool.tile([P, QT], mdt, tag=f"x{i}", name=f"x{i}") for i in range(IC)]
                for i in range(IC):
                    nc.sync.dma_start(wqk_sb[i][:, 0:LC], wqkT[i * P:(i + 1) * P, 0:LC])
                    nc.sync.dma_start(xt0[i][:], xT[i * P:(i + 1) * P, 0:QT])
                for i in range(IC):
                    nc.sync.dma_start(wqk_sb[i][:, LC:2 * LC],
                                      wqkT[i * P:(i + 1) * P, LC:2 * LC])
                for i in range(IC):
                    nc.sync.dma_start(wv_sb[i][:], wvT[i * P:(i + 1) * P, :])
                for tt in range(NQT):
                    if tt == 0:
                        xt = xt0
                    else:
                        xt = [xpool.tile([P, QT], mdt, tag=f"x{i}", name=f"x{i}") for i in range(IC)]
                        for i in range(IC):
                            nc.sync.dma_start(
                                xt[i][:], xT[i * P:(i + 1) * P, tt * QT:(tt + 1) * QT])
                    cols = slice(tt * QT, (tt + 1) * QT)
                    # qk-proj: psum[out-ch 128, tok 512] accumulated over in-ch
                    for oc in range(8):
                        ps = pspool.tile([P, QT], f32, tag="psA")
                        for i in range(IC):
                            nc.tensor.matmul(
                                ps[:], wqk_sb[i][:, oc * P:(oc + 1) * P],
                                xt[i][:], start=(i == 0), stop=(i == IC - 1))
                        if oc < 4:
                            nc.vector.tensor_scalar_add(
                                q_sb[oc][:, cols], ps[:], bqk_sb[:, oc:oc + 1])
                        else:
                            h0 = 2 * (oc - 4)
                            nc.vector.tensor_scalar_add(
                                kpad_sb[h0][0:D, cols], ps[0:D, :],
                                bqk_sb[0:D, oc:oc + 1])
                            nc.vector.tensor_scalar_add(
                                kpad_sb[h0 + 1][D:P, cols], ps[D:P, :],
                                bqk_sb[D:P, oc:oc + 1])
                    # v-proj: psum[tok 128, out-ch 512] per tok block
                    for tb in range(4):
                        kb = tt * 4 + tb
                        ps = pspool.tile([P, NH, D], f32, tag="psB")
                        for i in range(IC):
                            nc.tensor.matmul(
                                ps[:], xt[i][:, tb * P:(tb + 1) * P],
                                wv_sb[i][:], start=(i == 0), stop=(i == IC - 1))
                        v3d = v_sb[kb][:, 0:NH * VW].rearrange(
                            "p (h d) -> p h d", d=VW)
                        nc.scalar.activation(v3d[:, :, 0:D], ps[:], Copy)
                        # ones column for the softmax-denominator row of att@V
                        nc.scalar.activation(
                            v3d[:, :, D:D + 1], ps[:, :, 0:1],
                            Ident, bias=1.0, scale=0.0)

            # ---- Stage C: attention;  Stage D: c_proj (pipelined 1 qtt behind)
            with tc.tile_pool(name="wp", bufs=1) as wppool, \
                 tc.tile_pool(name="ybuf", bufs=1) as ypool, \
                 tc.tile_pool(name="att", bufs=4) as apool, \
                 tc.tile_pool(name="pss", bufs=2, space="PSUM") as ps_s_pool, \
                 tc.tile_pool(name="pso", bufs=2, space="PSUM") as ps_o_pool, \
                 tc.tile_pool(name="yraw", bufs=16) as yrawpool, \
                 tc.tile_pool(name="ptb", bufs=1, space="PSUM") as ps_b_pool, \
                 tc.tile_pool(name="psz", bufs=1, space="PSUM") as ps_z_pool, \
                 tc.tile_pool(name="zev", bufs=3) as zpool:
                wp_sb = [wppool.tile([P, C], mdt, tag=f"wp{i}", name=f"wp{i}") for i in range(4)]
                # y_sb: attention out, [local-ch 128, tok 2048] x 4 blocks
                y_sb = [ypool.tile([P, T], mdt, tag=f"y{i}", name=f"y{i}") for i in range(4)]
                for i in range(4):
                    nc.sync.dma_start(wp_sb[i][:], wpT[i * P:(i + 1) * P, :])
                nc.scalar.activation(ones_sb[:], wp_sb[0][:, 0:D],
                                     Ident, bias=1.0, scale=0.0)

                def recip_group(g):
                    nc.vector.reciprocal_approx_fast(rg_sb[g][:], sg_sb[g][:])
                    nc.vector.tensor_copy(rbf_sb[g][:], rg_sb[g][:])

                def norm_head(qtt, h, yraw):
                    # y = yraw * (1/rowsum), broadcast along partitions via a
                    # K=1 bf16 matmul
                    cols = slice(qtt * QT, (qtt + 1) * QT)
                    g, r0 = h // 3, 32 * (h % 3)
                    p0 = (h % 2) * D
                    btp = ps_b_pool.tile([P, QT], f32, tag="btp")
                    nc.tensor.matmul(
                        btp[0:D, :], ones_sb[r0:r0 + 1, 0:D],
                        rbf_sb[g][r0:r0 + 1, :],
                        start=True, stop=True)
                    nc.vector.tensor_mul(
                        y_sb[h // 2][p0:p0 + D, cols],
                        yraw[0:D, :], btp[0:D, :])

                def cproj(tt, pools):
                    cols = slice(tt * QT, (tt + 1) * QT)
                    for oc in range(8):
                        pool, tag = pools[oc % len(pools)]
                        ps = pool.tile([P, QT], f32, tag=tag)
                        for i in range(4):
                            nc.tensor.matmul(
                                ps[:], wp_sb[i][:, oc * P:(oc + 1) * P],
                                y_sb[i][:, cols], start=(i == 0), stop=(i == 3))
                        zt = zpool.tile([P, QT], mdt, tag="zt")
                        nc.scalar.activation(zt[:], ps[:], Ident,
                                             bias=bpj_sb[:, oc:oc + 1])
                        nc.sync.dma_start(
                            zT[oc * P:(oc + 1) * P, tt * QT:(tt + 1) * QT], zt[:])

                prev_yraws = None
                last = NQT - 1
                for qtt in range(NQT):
                    yraws = []
                    nkb = (qtt + 1) * 4

                    def evict_head(h, po):
                        # evict numerator+sum to SBUF, release the PSUM bank
                        yraw = yrawpool.tile([D + 1, QT], f32, tag="yraw")
                        nc.vector.tensor_copy(yraw[:], po[0:D + 1, :])
                        g, r0 = h // 3, 32 * (h % 3)
                        nc.vector.tensor_copy(sg_sb[g][r0:r0 + 1, :],
                                              yraw[D:D + 1, :])
                        yraws.append(yraw)
                        # pipelined prev-qtt epilogue: normalize after our
                        # first head (gives the DVE recip/cast time to land),
                        # c_proj after the second
                        if qtt > 0 and h == 0:
                            for hp in range(NH):
                                norm_head(qtt - 1, hp, prev_yraws[hp])
                        if qtt > 0 and h == 1:
                            cproj(qtt - 1, [(ps_z_pool, "pz")])
                        # last qtt: normalize each sum-group as soon as its
                        # heads are done, so the tail is only group 2 + c_proj
                        if qtt == last and h == 2:
                            recip_group(0)
                        if qtt == last and h == 5:
                            recip_group(1)
                        if qtt == last and h == 4:
                            for hp in (0, 1, 2):
                                norm_head(qtt, hp, yraws[hp])
                        if qtt == last and h == 6:
                            for hp in (3, 4, 5):
                                norm_head(qtt, hp, yraws[hp])

                    def attv(pend):
                        h, po, kbs, ns, c0s, os_, at = pend
                        for kb, n, c0, o in zip(kbs, ns, c0s, os_):
                            nc.tensor.matmul(
                                po[:, c0:QT], v_sb[kb][:, VW * h:VW * h + P],
                                at[:, o:o + n],
                                start=(kb == 0), stop=(kb == nkb - 1))
                        if kbs[-1] == nkb - 1:
                            evict_head(h, po)

                    # kb pairs share one PSUM tile + one exp; attV of pair i
                    # issues only after scores/exp of pair i+1 (crossing head
                    # boundaries) so the PE never stalls on the exp chain
                    pend = None
                    po = None
                    for h in range(NH):
                        for pi in range(nkb // 2):
                            if pi == 0:
                                po = ps_o_pool.tile([P, QT], f32, tag="po")
                            kbs = (2 * pi, 2 * pi + 1)
                            ns, c0s = [], []
                            for kb in kbs:
                                e = kb * P - qtt * QT
                                c0s.append(max(e, 0))
                                ns.append(QT - max(e, 0))
                            # pack both live column ranges into one tile; each
                            # matmul's output must stay inside one 512-col bank
                            o0 = 0
                            o1 = ns[0] if ns[0] + ns[1] <= QT else QT
                            width = o1 + ns[1]
                            ps = ps_s_pool.tile([P, 2 * QT], f32, tag="ps")
                            at = apool.tile([P, 2 * QT], mdt, tag="at")
                            for kb, n, c0, o in zip(kbs, ns, c0s, (o0, o1)):
                                nc.tensor.matmul(
                                    ps[:, o:o + n],
                                    kpad_sb[h][:, kb * P:(kb + 1) * P],
                                    q_sb[h // 2][:, qtt * QT + c0:(qtt + 1) * QT],
                                    start=True, stop=True)
                            nc.scalar.activation(at[:, 0:width], ps[:, 0:width],
                                                 Exp, scale=0.125)
                            for kb, n, c0, o in zip(kbs, ns, c0s, (o0, o1)):
                                if kb * P - qtt * QT >= 0:
                                    # zero strict upper triangle (never past
                                    # the first 128 live cols); on the idle
                                    # gpsimd so it can't queue behind DVE work
                                    m = min(n, P)
                                    nc.gpsimd.tensor_mul(at[:, o:o + m],
                                                         at[:, o:o + m],
                                                         mask_sb[:, 0:m])
                            if pend is not None:
                                attv(pend)
                            pend = (h, po, kbs, ns, c0s, (o0, o1), at)
                    attv(pend)
                    if qtt < last:
                        for g in range(3):
                            recip_group(g)
                    prev_yraws = yraws
                recip_group(2)
                for hp in (6, 7):
                    norm_head(last, hp, prev_yraws[hp])
                cproj(last, [(ps_z_pool, "pz"), (ps_b_pool, "btp")])
    nc.compile()
    return nc


def get_nc():
    if "nc" not in _nc_cache:
        _nc_cache["nc"] = _build_nc()
    return _nc_cache["nc"]


def _mm_np_dtype():
    if MM_DT == "bfloat16":
        import ml_dtypes
        return np.dtype(ml_dtypes.bfloat16)
    return np.dtype(np.float32)


def make_in_maps(x, Wqkv, bqkv, Wproj, bproj):
    x = np.asarray(x, np.float32)
    Wqkv = np.asarray(Wqkv, np.float32)
    bqkv = np.asarray(bqkv, np.float32)
    Wproj = np.asarray(Wproj, np.float32)
    bproj = np.asarray(bproj, np.float32)
    Wq, Wk, Wv = Wqkv[0:C], Wqkv[C:2 * C], Wqkv[2 * C:3 * C]
    bq, bk, bv = bqkv[0:C], bqkv[C:2 * C], bqkv[2 * C:3 * C]
    mdt = _mm_np_dtype()
    mask = np.triu(np.ones((P, P), np.float32)).astype(mdt)
    in_maps = []
    for b in range(B):
        xTb = np.ascontiguousarray(x[b].T.astype(mdt))
        for s in range(2):
            cols = slice(s * LC, (s + 1) * LC)
            wqkT = np.ascontiguousarray(
                np.concatenate([Wq[cols], Wk[cols]], 0).T.astype(mdt))
            bqk_ = np.concatenate([bq[cols], bk[cols]])
            wvT_ = np.ascontiguousarray(Wv[cols].T.astype(mdt))
            wpT_ = np.ascontiguousarray(Wproj[:, cols].T.astype(mdt))
            bp_eff = bv[cols] @ Wproj[:, cols].T
            if s == 0:
                bp_eff = bp_eff + bproj
            in_maps.append({
                "xT": xTb,
                "wqkT": wqkT,
                "bqk": np.ascontiguousarray(bqk_.reshape(8, P).T),
                "wvT": wvT_,
                "wpT": wpT_,
                "bpj": np.ascontiguousarray(bp_eff.astype(np.float32).reshape(8, P).T),
                "maskT": mask,
            })
    return in_maps


def gather_out(results):
    out = np.empty((B, T, C), np.float32)
    for b in range(B):
        zt = (results[2 * b]["zT"].astype(np.float32)
              + results[2 * b + 1]["zT"].astype(np.float32))
        out[b] = zt.T
    return out


def kernel(x, Wqkv, bqkv, Wproj, bproj):
    from concourse.bass_utils import run_bass_kernel_spmd

    in_maps = make_in_maps(x, Wqkv, bqkv, Wproj, bproj)
    try:
        res = run_bass_kernel_spmd(get_nc(), in_maps, core_ids=list(range(8)))
    except Exception:
        # transient device faults have been observed once; retry a single time
        res = run_bass_kernel_spmd(get_nc(), in_maps, core_ids=list(range(8)))
    return gather_out(res.results)
